# revision 15
# baseline (speedup 1.0000x reference)
"""Multi-head self-attention block (B=2, N=2048, C=1024, H=16, D=64) + output
projection, sharded over 8 Trainium2 NeuronCores.

Sharding: core c handles batch b=c//4 and heads 4*(c%4)..4*(c%4)+3 (data +
head parallel).  The output projection is row-sharded over the input-channel
dim (each core multiplies its 256 attention channels into a full [N, 1024]
partial product); the 4 partials per batch are summed on the host (the
"all-reduce") and the bias is added there.

Device kernel layout (per core, fp32 scores / bf16 probs):
  - q, k fed pre-transposed per head-pair: [128, N] tiles whose partition dim
    stacks the two heads' 64 attention dims; K=64 QK matmuls of the two heads
    run in disjoint PE row groups and overlap.
  - softmax exp is SPLIT across two engines: ScalarE (ACT) computes true
    exp on query-columns [0:SA]; VectorE (DVE) computes a Schraudolph-style
    exp on [SA:1024]: i16 = round_i16(x*128/ln2 + (127*128-c)) written
    through an int16 bitcast view of the bf16 probs tile -- the int16 bit
    pattern IS the bf16 exp approximation (max rel err ~3%, verified on HW).
    This halves the former single-engine exp bottleneck.
  - AV: lhsT is v augmented with a ones column, so PSUM accumulates x^T
    unnormalized and the softmax denominator in the same accumulation group.
  - normalization: PSUM evacuated by DMA (x^T rows -> xu, denom rows -> dn),
    denominators partition-broadcast by SBUF->SBUF DMA, then reciprocal +
    multiply on DVE, deferred into the next pair's loop so the DMA latency
    never stalls the DVE queue.
  - projection: x^T chunks are the matmul lhsT directly; [N,256]@[256,1024]
    partial product is written out unreduced.
"""

import os
from contextlib import ExitStack

import ml_dtypes
import numpy as np

import concourse.bass as bass
import concourse.tile as tile
from concourse import bacc, mybir
from concourse._compat import with_exitstack
from concourse import bass_utils

F32 = mybir.dt.float32
I16 = mybir.dt.int16

B, N, C, H, D = 2, 2048, 1024, 16, 64
NCORES = 8
HPC = 4  # heads per core
NPAIR = HPC // 2

# Schraudolph exp-to-bf16 constants: bf16 bits = round_i16(x*SCHR_A + SCHR_B)
_LN2 = 0.6931471805599453
SCHR_A = 128.0 / _LN2
SCHR_C = 5.6  # calibrated: balances the piecewise-linear 2^f error
SCHR_B = 127.0 * 128.0 - SCHR_C


def _mm_dtypes():
    """PE dtypes: qk/proj "f32" (exact, 4 cyc/col) or "f32r" (1 cyc/col);
    av bf16 (1 cyc/col, keeps the PE HAM clock gate warmer than f32r)."""
    qk = os.environ.get("ATTN_KERNEL_QK_DT", "f32r")
    av = os.environ.get("ATTN_KERNEL_AV_DT", "bf16")
    pj = os.environ.get("ATTN_KERNEL_PJ_DT", "f16")
    m = {
        "f32": F32,
        "f32r": mybir.dt.float32r,
        "bf16": mybir.dt.bfloat16,
        "f16": mybir.dt.float16,
    }
    return m[qk], m[av], m[pj]


def _bcast_row(row_ap, nparts):
    """AP view replicating a 1-partition row across `nparts` partitions."""
    return bass.AP(
        tensor=row_ap.tensor,
        offset=row_ap.offset,
        ap=[[0, nparts], *row_ap.ap],
    )


@with_exitstack
def attention_body(ctx: ExitStack, tc: tile.TileContext, out, qt, kt, vp, wt):
    """Emit the per-core attention+projection program.

    APs:
      out  [N, OW]          partial projection output (fp32)
      qt   [NPAIR, 128, N]  q transposed, head pair stacked on partitions
      kt   [NPAIR, 128, N]  k transposed, same packing
      vp   [2*NPAIR, 128, NJ, 128]  v chunks as AV lhsT: for even heads v in
           cols 0:64 and ones in col 64; for odd heads v in cols 64:128 and
           ones in col 32 (so x^T lands on the partitions matching qt packing)
      wt   [NPAIR, 128, OW] proj_w slice, transposed to [channel, out]
    """
    nc = tc.nc
    P = 128
    npair, _, n = qt.shape
    NJ = n // P          # key chunks
    HW = n // 2          # query half processed per pair loop
    NT = max(1, HW // 512)
    MS = HW // NT        # matmul free-dim chunk (<=512, one PSUM bank)
    OW = wt.shape[2]
    OT = max(1, OW // 512)
    OS = OW // OT
    SA = int(os.environ.get("ATTN_SPLIT", "640"))      # ACT cols [0:SA]
    WJ = int(os.environ.get("ATTN_WARM_EVERY", "2"))   # hot-QK cadence
    WCOLS = int(os.environ.get("ATTN_WARM_COLS", "512"))
    nwarm = int(os.environ.get("ATTN_KERNEL_WARMUP", "8"))
    nburst = int(os.environ.get("ATTN_KERNEL_REWARM", "1"))
    CH = 512             # input DMA chunk width

    sing = ctx.enter_context(tc.tile_pool(name="sing", bufs=1))
    probs_pool = ctx.enter_context(tc.tile_pool(name="probs", bufs=6))
    work = ctx.enter_context(tc.tile_pool(name="work", bufs=2))
    ost = ctx.enter_context(tc.tile_pool(name="ost", bufs=3))
    psum = ctx.enter_context(tc.tile_pool(name="psum", bufs=2, space="PSUM"))
    dram = ctx.enter_context(tc.tile_pool(name="dram", bufs=2, space="DRAM"))

    # HAM warm-up: plain-fp32 matmuls lift the PE clock gate to 2.4 GHz;
    # periodic small bursts inside the j-loop keep it there.
    wtile = None
    if nwarm or nburst or WJ:
        wtile = sing.tile([P, 512], F32, tag="warm", name="warm")
        nc.vector.memset(wtile, 1.0)

    def warm_burst(count, cols=512, name="w"):
        for w in range(count):
            pw = psum.tile([P, cols], F32, tag="ps", name=f"{name}{w}")
            nc.tensor.matmul(
                pw, lhsT=wtile[:, 0:128], rhs=wtile[:, 0:cols], start=True, stop=True
            )

    if nwarm:
        warm_burst(nwarm, 512, "warmps")

    # --- input tiles + chunked DMAs (ordered so pair-0 work starts early) ---
    qts, kts, wts, xts, vps = [], [], [], [], []
    for p in range(npair):
        qts.append(sing.tile([P, n], qt.dtype, tag=f"qt{p}", name=f"qts{p}"))
        kts.append(sing.tile([P, n], kt.dtype, tag=f"kt{p}", name=f"kts{p}"))
        wts.append(sing.tile([P, OW], wt.dtype, tag=f"wt{p}", name=f"wts{p}"))
        xts.append(sing.tile([P, n], wt.dtype, tag=f"xt{p}", name=f"xts{p}"))
    for h in range(2 * npair):
        vps.append(sing.tile([P, NJ, P], vp.dtype, tag=f"vp{h}", name=f"vps{h}"))

    NCH = n // CH
    VCH = NJ // NCH  # v chunks per input chunk
    for p in range(npair):
        for c in range(NCH):
            cs = slice(c * CH, (c + 1) * CH)
            nc.sync.dma_start(kts[p][:, cs], kt[p][:, cs])
            if c < 2:  # half-0 query cols first; rest after k/v
                nc.sync.dma_start(qts[p][:, cs], qt[p][:, cs])
            js = slice(c * VCH, (c + 1) * VCH)
            nc.sync.dma_start(vps[2 * p][:, js, :], vp[2 * p][:, js, :])
            nc.sync.dma_start(vps[2 * p + 1][:, js, :], vp[2 * p + 1][:, js, :])
        for c in range(2, NCH):
            cs = slice(c * CH, (c + 1) * CH)
            nc.sync.dma_start(qts[p][:, cs], qt[p][:, cs])
    for p in range(npair):
        nc.sync.dma_start(wts[p], wt[p])

    pending_normalize = []

    def emit_pending():
        while pending_normalize:
            pending_normalize.pop(0)()

    for Hi in range(2):
        h0 = Hi * HW
        for p in range(npair):
            po = [
                psum.tile([P, HW], F32, tag="po", name=f"po{Hi}{p}{a}")
                for a in range(2)
            ]

            def emit_qk(j):
                # QK for both heads; a0's two matmuls first (so exp(j+1)a0
                # unblocks earliest), a1's overlap them in the disjoint PE
                # row group.  Every WJth chunk the t1 matmuls run in plain
                # fp32 (bitcast of the same data): ~2x cost for that matmul,
                # but it registers as PE activity in the HAM clock gate --
                # f32r/bf16 streaming does not -- keeping the PE at 2.4 GHz.
                pss = [
                    psum.tile([P, HW], F32, tag="ps", name=f"ps{Hi}{p}{j}{a}")
                    for a in range(2)
                ]
                hot = WJ and (j % WJ == WJ - 1)
                for a in range(2):
                    rows = slice(a * 64, a * 64 + 64)
                    for t in range(NT):
                        lhsT = kts[p][rows, j * P : (j + 1) * P]
                        rhs = qts[p][rows, h0 + t * MS : h0 + (t + 1) * MS]
                        # hot matmul on a1 only: a0 gates the exp pipeline,
                        # a1 has slack (exp a1 starts one ACT-tile later)
                        if hot and t == NT - 1 and a == 1:
                            lhsT = lhsT.bitcast(F32)
                            rhs = rhs.bitcast(F32)
                        nc.tensor.matmul(
                            pss[a][:, t * MS : (t + 1) * MS],
                            lhsT=lhsT,
                            rhs=rhs,
                            start=True,
                            stop=True,
                        )
                return pss

            pss = emit_qk(0)
            for j in range(NJ):
                pbs = []
                for a in range(2):
                    pb = probs_pool.tile(
                        [P, HW], vp.dtype, tag="pb", name=f"pb{Hi}{p}{j}{a}"
                    )
                    if SA > 0:
                        nc.scalar.activation(
                            pb[:, 0:SA],
                            pss[a][:, 0:SA],
                            mybir.ActivationFunctionType.Exp,
                        )
                    if SA < HW:
                        nc.vector.tensor_scalar(
                            pb[:, SA:HW].bitcast(I16),
                            pss[a][:, SA:HW],
                            SCHR_A,
                            SCHR_B,
                            mybir.AluOpType.mult,
                            mybir.AluOpType.add,
                        )
                    pbs.append(pb)
                if j + 1 < NJ:
                    pss = emit_qk(j + 1)
                if j == 2:
                    emit_pending()
                for a in range(2):
                    for t in range(NT):
                        nc.tensor.matmul(
                            po[a][:, t * MS : (t + 1) * MS],
                            lhsT=vps[2 * p + a][:, j, :],
                            rhs=pbs[a][:, t * MS : (t + 1) * MS],
                            start=(j == 0),
                            stop=(j == NJ - 1),
                        )
            # Evacuate PSUM by DMA (x^T rows + denominator rows), broadcast
            # the denominators across partitions, then normalize on DVE.
            # The recip+mul are deferred into the next pair's j-loop so the
            # DMA latency never blocks the DVE queue head.
            # Evacuate PSUM in two engine copies; the denominator rows ride
            # along (xuA row 64 = even head's denom, xuB row 32 = odd's).
            xuA = work.tile([65, HW], F32, tag="xuA", name=f"xuA{Hi}{p}")
            xuB = work.tile([P, HW], F32, tag="xuB", name=f"xuB{Hi}{p}")
            nc.scalar.copy(xuA, po[0][0:65, :])
            nc.vector.tensor_copy(xuB[64:128, :], po[1][64:128, :])
            nc.vector.tensor_copy(xuB[32:33, :], po[1][32:33, :])
            dsc = dram.tile([2, HW], F32, tag="dsc", name=f"dsc{Hi}{p}")
            nc.sync.dma_start(dsc[0:1, :], xuA[64:65, :])
            nc.sync.dma_start(dsc[1:2, :], xuB[32:33, :])
            rbd = work.tile([P, HW], F32, tag="rbd", name=f"rbd{Hi}{p}")
            nc.sync.dma_start(rbd[0:64, :], _bcast_row(dsc[0], 64))
            nc.sync.dma_start(rbd[64:128, :], _bcast_row(dsc[1], 64))
            if nburst:
                warm_burst(nburst, 512, f"rb{Hi}{p}")

            def normalize(p=p, h0=h0, Hi=Hi, xuA=xuA, xuB=xuB, rbd=rbd):
                rb = work.tile([P, HW], F32, tag="rb", name=f"rb{Hi}{p}")
                rscr = work.tile([P, HW], F32, tag="rscr", name=f"rscr{Hi}{p}")
                nc.vector.reciprocal_approx_accurate(rb, rbd, rscr)
                nc.vector.tensor_mul(xts[p][0:64, h0 : h0 + HW], xuA[0:64, :], rb[0:64, :])
                nc.vector.tensor_mul(xts[p][64:128, h0 : h0 + HW], xuB[64:128, :], rb[64:128, :])

            pending_normalize.append(normalize)
    emit_pending()

    # projection: emitted after all attention so attention work is always
    # available behind it in the PE queue
    for i in range(n // P):
        pp = psum.tile([P, OW], F32, tag="ps", name=f"pp{i}")
        for cc in range(npair):
            for t in range(OT):
                nc.tensor.matmul(
                    pp[:, t * OS : (t + 1) * OS],
                    lhsT=xts[cc][:, i * P : (i + 1) * P],
                    rhs=wts[cc][:, t * OS : (t + 1) * OS],
                    start=(cc == 0),
                    stop=(cc == npair - 1),
                )
        ot = ost.tile([P, OW], F32, tag="ot", name=f"ot{i}")
        if i % 2 == 0:
            nc.vector.tensor_copy(ot, pp)
        else:
            nc.scalar.copy(ot, pp)
        nc.sync.dma_start(out[i * P : (i + 1) * P, :], ot)


def build_module(n=N, ow=C, npair=NPAIR):
    qkd, avd, pjd = _mm_dtypes()
    nc = bacc.Bacc("TRN2", target_bir_lowering=False, debug=False, num_devices=NCORES)
    nj = n // 128
    qt = nc.dram_tensor("qt", [npair, 128, n], qkd, kind="ExternalInput")
    kt = nc.dram_tensor("kt", [npair, 128, n], qkd, kind="ExternalInput")
    vp = nc.dram_tensor("vp", [2 * npair, 128, nj, 128], avd, kind="ExternalInput")
    wt = nc.dram_tensor("wt", [npair, 128, ow], pjd, kind="ExternalInput")
    out = nc.dram_tensor("out", [n, ow], F32, kind="ExternalOutput")
    with tile.TileContext(nc) as tc:
        attention_body(tc, out.ap(), qt.ap(), kt.ap(), vp.ap(), wt.ap())
    nc.compile()
    return nc


def shard_inputs(q, k, v, proj_w):
    """Build the 8 per-core input maps from the full tensors."""
    q = np.asarray(q, dtype=np.float32)
    k = np.asarray(k, dtype=np.float32)
    v = np.asarray(v, dtype=np.float32)
    proj_w = np.asarray(proj_w, dtype=np.float32)
    b_, n_, c_ = q.shape
    h_ = k.shape[1]
    d_ = c_ // h_
    nj = n_ // 128
    _np_dt = {"f32": np.float32, "f32r": np.float32, "bf16": ml_dtypes.bfloat16,
              "f16": np.float16}
    qk_np = _np_dt[os.environ.get("ATTN_KERNEL_QK_DT", "f32r")]
    qh = np.ascontiguousarray(
        q.reshape(b_, n_, h_, d_).transpose(0, 2, 3, 1).astype(qk_np)
    )
    kh = np.ascontiguousarray(k.transpose(0, 1, 3, 2).astype(qk_np))
    in_maps = []
    for c in range(NCORES):
        b = c // 4
        hh0 = HPC * (c % 4)
        qt = np.ascontiguousarray(qh[b, hh0 : hh0 + HPC].reshape(NPAIR, 128, n_))
        kt = np.ascontiguousarray(kh[b, hh0 : hh0 + HPC].reshape(NPAIR, 128, n_))
        avd = os.environ.get("ATTN_KERNEL_AV_DT", "bf16")
        vp_np = ml_dtypes.bfloat16 if avd == "bf16" else np.float32
        vp = np.zeros((HPC, 128, nj, 128), vp_np)
        for hh in range(HPC):
            vv = v[b, hh0 + hh].reshape(nj, 128, d_).transpose(1, 0, 2)
            if hh % 2 == 0:
                vp[hh][:, :, 0:64] = vv
                vp[hh][:, :, 64] = 1.0
            else:
                vp[hh][:, :, 64:128] = vv
                vp[hh][:, :, 32] = 1.0
        ch0 = hh0 * d_
        pj_np = _np_dt[os.environ.get("ATTN_KERNEL_PJ_DT", "f16")]
        wt = np.ascontiguousarray(
            proj_w[:, ch0 : ch0 + HPC * d_].T.reshape(NPAIR, 128, c_).astype(pj_np)
        )
        in_maps.append({"qt": qt, "kt": kt, "vp": vp, "wt": wt})
    return in_maps


def reduce_outputs(results, proj_b):
    """Sum the per-core partial projections per batch and add the bias."""
    outs = [np.asarray(r["out"], dtype=np.float32) for r in results]
    full = np.stack(
        [outs[0] + outs[1] + outs[2] + outs[3], outs[4] + outs[5] + outs[6] + outs[7]]
    )
    return (full + np.asarray(proj_b, dtype=np.float32)[None, None, :]).astype(
        np.float32
    )


_NC_CACHE = {}


def _get_module():
    if "nc" not in _NC_CACHE:
        _NC_CACHE["nc"] = build_module()
    return _NC_CACHE["nc"]


def kernel(q, k, v, proj_w, proj_b):
    nc = _get_module()
    in_maps = shard_inputs(q, k, v, proj_w)
    trace = bool(int(os.environ.get("ATTN_KERNEL_TRACE", "0")))
    kwargs = {}
    tmpdir = os.environ.get("ATTN_KERNEL_TMPDIR")
    if trace and tmpdir:
        os.makedirs(tmpdir, exist_ok=True)
        kwargs["tmpdir"] = tmpdir
    res = bass_utils.run_bass_kernel_spmd(
        nc, in_maps, core_ids=list(range(NCORES)), trace=trace, **kwargs
    )
    if trace:
        _NC_CACHE["last_results"] = res
    return reduce_outputs(res.results, proj_b)


# revision 21
# speedup vs baseline: 1.1390x; 1.1390x over previous
"""Multi-head self-attention block (B=2, N=2048, C=1024, H=16, D=64) + output
projection, sharded over 8 Trainium2 NeuronCores.

Sharding: core c handles batch b=c//4 and heads 4*(c%4)..4*(c%4)+3 (data +
head parallel).  The output projection is row-sharded over the input-channel
dim (each core multiplies its 256 attention channels into a full [N, 1024]
partial product); the 4 partials per batch are summed on the host (the
"all-reduce") and the bias is added there.

Device kernel layout (per core, fp32 scores / bf16 probs):
  - q, k fed pre-transposed per head-pair: [128, N] tiles whose partition dim
    stacks the two heads' 64 attention dims; K=64 QK matmuls of the two heads
    run in disjoint PE row groups and overlap.
  - softmax exp is SPLIT across two engines: ScalarE (ACT) computes true
    exp on query-columns [0:SA]; VectorE (DVE) computes a Schraudolph-style
    exp on [SA:1024]: i16 = round_i16(x*128/ln2 + (127*128-c)) written
    through an int16 bitcast view of the bf16 probs tile -- the int16 bit
    pattern IS the bf16 exp approximation (max rel err ~3%, verified on HW).
    This halves the former single-engine exp bottleneck.
  - AV: lhsT is v augmented with a ones column, so PSUM accumulates x^T
    unnormalized and the softmax denominator in the same accumulation group.
  - normalization: PSUM evacuated by DMA (x^T rows -> xu, denom rows -> dn),
    denominators partition-broadcast by SBUF->SBUF DMA, then reciprocal +
    multiply on DVE, deferred into the next pair's loop so the DMA latency
    never stalls the DVE queue.
  - projection: x^T chunks are the matmul lhsT directly; [N,256]@[256,1024]
    partial product is written out unreduced.
"""

import os
from contextlib import ExitStack

import ml_dtypes
import numpy as np

import concourse.bass as bass
import concourse.tile as tile
from concourse import bacc, mybir
from concourse._compat import with_exitstack
from concourse import bass_utils

F32 = mybir.dt.float32
I16 = mybir.dt.int16

B, N, C, H, D = 2, 2048, 1024, 16, 64
NCORES = 8
HPC = 4  # heads per core
NPAIR = HPC // 2

# Schraudolph exp-to-bf16 constants: bf16 bits = round_i16(x*SCHR_A + SCHR_B)
_LN2 = 0.6931471805599453
SCHR_A = 128.0 / _LN2
SCHR_C = 5.6  # calibrated: balances the piecewise-linear 2^f error
SCHR_B = 127.0 * 128.0 - SCHR_C


def _mm_dtypes():
    """PE dtypes: qk/proj "f32" (exact, 4 cyc/col) or "f32r" (1 cyc/col);
    av bf16 (1 cyc/col, keeps the PE HAM clock gate warmer than f32r)."""
    qk = os.environ.get("ATTN_KERNEL_QK_DT", "f16")
    av = os.environ.get("ATTN_KERNEL_AV_DT", "bf16")
    pj = os.environ.get("ATTN_KERNEL_PJ_DT", "f16")
    m = {
        "f32": F32,
        "f32r": mybir.dt.float32r,
        "bf16": mybir.dt.bfloat16,
        "f16": mybir.dt.float16,
    }
    return m[qk], m[av], m[pj]


def _bcast_row(row_ap, nparts):
    """AP view replicating a 1-partition row across `nparts` partitions."""
    return bass.AP(
        tensor=row_ap.tensor,
        offset=row_ap.offset,
        ap=[[0, nparts], *row_ap.ap],
    )


@with_exitstack
def attention_body(ctx: ExitStack, tc: tile.TileContext, out, qt, kt, vp, wt):
    """Emit the per-core attention+projection program.

    APs:
      out  [N, OW]          partial projection output (fp32)
      qt   [NPAIR, 128, N]  q transposed, head pair stacked on partitions
      kt   [NPAIR, 128, N]  k transposed, same packing
      vp   [2*NPAIR, 128, NJ, 128]  v chunks as AV lhsT: for even heads v in
           cols 0:64 and ones in col 64; for odd heads v in cols 64:128 and
           ones in col 32 (so x^T lands on the partitions matching qt packing)
      wt   [NPAIR, 128, OW] proj_w slice, transposed to [channel, out]
    """
    nc = tc.nc
    P = 128
    npair, _, n = qt.shape
    NJ = n // P          # key chunks
    HW = n // 2          # query half processed per pair loop
    NT = max(1, HW // 512)
    MS = HW // NT        # matmul free-dim chunk (<=512, one PSUM bank)
    OW = wt.shape[2]
    OT = max(1, OW // 512)
    OS = OW // OT
    SA = int(os.environ.get("ATTN_SPLIT", "640"))      # ACT cols [0:SA]
    WJ = int(os.environ.get("ATTN_WARM_EVERY", "1"))   # idle-filler cadence
    WCOLS = int(os.environ.get("ATTN_WARM_COLS", "256"))
    WDT = os.environ.get("ATTN_WARM_DT", "f32")        # filler dtype
    nwarm = int(os.environ.get("ATTN_KERNEL_WARMUP", "8"))
    nburst = int(os.environ.get("ATTN_KERNEL_REWARM", "1"))
    CH = 512             # input DMA chunk width

    sing = ctx.enter_context(tc.tile_pool(name="sing", bufs=1))
    probs_pool = ctx.enter_context(tc.tile_pool(name="probs", bufs=6))
    work = ctx.enter_context(tc.tile_pool(name="work", bufs=2))
    ost = ctx.enter_context(tc.tile_pool(name="ost", bufs=3))
    psum = ctx.enter_context(tc.tile_pool(name="psum", bufs=2, space="PSUM"))
    dram = ctx.enter_context(tc.tile_pool(name="dram", bufs=2, space="DRAM"))

    # HAM warm-up: plain-fp32 matmuls lift the PE clock gate to 2.4 GHz;
    # periodic small bursts inside the j-loop keep it there.
    wtile = None
    if nwarm or nburst or WJ:
        wtile = sing.tile([P, 512], F32, tag="warm", name="warm")
        nc.vector.memset(wtile, 1.0)
    if WDT == "f32":
        warm_lhsT, warm_rhs = wtile[:, 0:128], wtile
    else:
        wtb = sing.tile([P, 512], mybir.dt.bfloat16, tag="warmb", name="warmb")
        nc.vector.memset(wtb, 1.0)
        warm_lhsT, warm_rhs = wtb[:, 0:128], wtb

    def warm_burst(count, cols=512, name="w"):
        for w in range(count):
            pw = psum.tile([P, cols], F32, tag="ps", name=f"{name}{w}")
            nc.tensor.matmul(
                pw, lhsT=wtile[:, 0:128], rhs=wtile[:, 0:cols], start=True, stop=True
            )

    if nwarm:
        warm_burst(nwarm, 512, "warmps")

    # --- input tiles + chunked DMAs (ordered so pair-0 work starts early) ---
    qts, kts, wts, xts, vps = [], [], [], [], []
    for p in range(npair):
        qts.append(sing.tile([P, n], qt.dtype, tag=f"qt{p}", name=f"qts{p}"))
        kts.append(sing.tile([P, n], kt.dtype, tag=f"kt{p}", name=f"kts{p}"))
        wts.append(sing.tile([P, OW], wt.dtype, tag=f"wt{p}", name=f"wts{p}"))
        xts.append(sing.tile([P, n], wt.dtype, tag=f"xt{p}", name=f"xts{p}"))
    for h in range(2 * npair):
        vps.append(sing.tile([P, NJ, P], vp.dtype, tag=f"vp{h}", name=f"vps{h}"))

    NCH = n // CH
    VCH = NJ // NCH  # v chunks per input chunk
    for p in range(npair):
        for c in range(NCH):
            cs = slice(c * CH, (c + 1) * CH)
            nc.sync.dma_start(kts[p][:, cs], kt[p][:, cs])
            if c < 2:  # half-0 query cols first; rest after k/v
                nc.sync.dma_start(qts[p][:, cs], qt[p][:, cs])
            js = slice(c * VCH, (c + 1) * VCH)
            nc.sync.dma_start(vps[2 * p][:, js, :], vp[2 * p][:, js, :])
            nc.sync.dma_start(vps[2 * p + 1][:, js, :], vp[2 * p + 1][:, js, :])
        for c in range(2, NCH):
            cs = slice(c * CH, (c + 1) * CH)
            nc.sync.dma_start(qts[p][:, cs], qt[p][:, cs])
    for p in range(npair):
        nc.sync.dma_start(wts[p], wt[p])

    pending_normalize = []

    def emit_pending():
        while pending_normalize:
            pending_normalize.pop(0)()

    for Hi in range(2):
        h0 = Hi * HW
        for p in range(npair):
            po = [
                psum.tile([P, HW], F32, tag="po", name=f"po{Hi}{p}{a}")
                for a in range(2)
            ]

            def emit_qk(j):
                # QK for both heads; a0's two matmuls first (so exp(j+1)a0
                # unblocks earliest), a1's overlap them in the disjoint PE
                # row group.
                pss = [
                    psum.tile([P, HW], F32, tag="ps", name=f"ps{Hi}{p}{j}{a}")
                    for a in range(2)
                ]
                for a in range(2):
                    rows = slice(a * 64, a * 64 + 64)
                    for t in range(NT):
                        nc.tensor.matmul(
                            pss[a][:, t * MS : (t + 1) * MS],
                            lhsT=kts[p][rows, j * P : (j + 1) * P],
                            rhs=qts[p][rows, h0 + t * MS : h0 + (t + 1) * MS],
                            start=True,
                            stop=True,
                        )
                return pss

            pss = emit_qk(0)
            for j in range(NJ):
                pbs = []
                for a in range(2):
                    pb = probs_pool.tile(
                        [P, HW], vp.dtype, tag="pb", name=f"pb{Hi}{p}{j}{a}"
                    )
                    if SA > 0:
                        nc.scalar.activation(
                            pb[:, 0:SA],
                            pss[a][:, 0:SA],
                            mybir.ActivationFunctionType.Exp,
                        )
                    if SA < HW:
                        nc.vector.tensor_scalar(
                            pb[:, SA:HW].bitcast(I16),
                            pss[a][:, SA:HW],
                            SCHR_A,
                            SCHR_B,
                            mybir.AluOpType.mult,
                            mybir.AluOpType.add,
                        )
                    pbs.append(pb)
                pss_cur = pss
                if j + 1 < NJ:
                    pss = emit_qk(j + 1)
                if j == 2:
                    emit_pending()
                for a in range(2):
                    for t in range(NT):
                        nc.tensor.matmul(
                            po[a][:, t * MS : (t + 1) * MS],
                            lhsT=vps[2 * p + a][:, j, :],
                            rhs=pbs[a][:, t * MS : (t + 1) * MS],
                            start=(j == 0),
                            stop=(j == NJ - 1),
                        )
                # Idle-filler warm matmul: writes into the just-consumed a1
                # score tile (dead data; the ring overwrites it with
                # start=True two chunks later).  Runs in the PE's natural
                # idle window after the AVs, keeping the HAM activity
                # monitor busy so the clock gate stays at 2.4 GHz.
                if WJ and (j % WJ == WJ - 1):
                    nc.tensor.matmul(
                        pss_cur[1][:, MS : MS + WCOLS],
                        lhsT=warm_lhsT,
                        rhs=warm_rhs[:, 0:WCOLS],
                        start=True,
                        stop=True,
                    )
            # Evacuate PSUM by DMA (x^T rows + denominator rows), broadcast
            # the denominators across partitions, then normalize on DVE.
            # The recip+mul are deferred into the next pair's j-loop so the
            # DMA latency never blocks the DVE queue head.
            # Evacuate PSUM in two engine copies; the denominator rows ride
            # along (xuA row 64 = even head's denom, xuB row 32 = odd's).
            xuA = work.tile([65, HW], F32, tag="xuA", name=f"xuA{Hi}{p}")
            xuB = work.tile([P, HW], F32, tag="xuB", name=f"xuB{Hi}{p}")
            nc.scalar.copy(xuA, po[0][0:65, :])
            nc.vector.tensor_copy(xuB[64:128, :], po[1][64:128, :])
            nc.vector.tensor_copy(xuB[32:33, :], po[1][32:33, :])
            dsc = dram.tile([2, HW], F32, tag="dsc", name=f"dsc{Hi}{p}")
            nc.sync.dma_start(dsc[0:1, :], xuA[64:65, :])
            nc.sync.dma_start(dsc[1:2, :], xuB[32:33, :])
            rbd = work.tile([P, HW], F32, tag="rbd", name=f"rbd{Hi}{p}")
            nc.sync.dma_start(rbd[0:64, :], _bcast_row(dsc[0], 64))
            nc.sync.dma_start(rbd[64:128, :], _bcast_row(dsc[1], 64))
            if nburst:
                warm_burst(nburst, 512, f"rb{Hi}{p}")

            def normalize(p=p, h0=h0, Hi=Hi, xuA=xuA, xuB=xuB, rbd=rbd):
                rb = work.tile([P, HW], F32, tag="rb", name=f"rb{Hi}{p}")
                rscr = work.tile([P, HW], F32, tag="rscr", name=f"rscr{Hi}{p}")
                nc.vector.reciprocal_approx_accurate(rb, rbd, rscr)
                nc.vector.tensor_mul(xts[p][0:64, h0 : h0 + HW], xuA[0:64, :], rb[0:64, :])
                nc.vector.tensor_mul(xts[p][64:128, h0 : h0 + HW], xuB[64:128, :], rb[64:128, :])

            pending_normalize.append(normalize)
    emit_pending()

    # projection: emitted after all attention so attention work is always
    # available behind it in the PE queue
    for i in range(n // P):
        pp = psum.tile([P, OW], F32, tag="ps", name=f"pp{i}")
        for cc in range(npair):
            for t in range(OT):
                nc.tensor.matmul(
                    pp[:, t * OS : (t + 1) * OS],
                    lhsT=xts[cc][:, i * P : (i + 1) * P],
                    rhs=wts[cc][:, t * OS : (t + 1) * OS],
                    start=(cc == 0),
                    stop=(cc == npair - 1),
                )
        ot = ost.tile([P, OW], F32, tag="ot", name=f"ot{i}")
        if i % 2 == 0:
            nc.vector.tensor_copy(ot, pp)
        else:
            nc.scalar.copy(ot, pp)
        nc.sync.dma_start(out[i * P : (i + 1) * P, :], ot)


def build_module(n=N, ow=C, npair=NPAIR):
    qkd, avd, pjd = _mm_dtypes()
    nc = bacc.Bacc("TRN2", target_bir_lowering=False, debug=False, num_devices=NCORES)
    nj = n // 128
    qt = nc.dram_tensor("qt", [npair, 128, n], qkd, kind="ExternalInput")
    kt = nc.dram_tensor("kt", [npair, 128, n], qkd, kind="ExternalInput")
    vp = nc.dram_tensor("vp", [2 * npair, 128, nj, 128], avd, kind="ExternalInput")
    wt = nc.dram_tensor("wt", [npair, 128, ow], pjd, kind="ExternalInput")
    out = nc.dram_tensor("out", [n, ow], F32, kind="ExternalOutput")
    with tile.TileContext(nc) as tc:
        attention_body(tc, out.ap(), qt.ap(), kt.ap(), vp.ap(), wt.ap())
    nc.compile()
    return nc


def shard_inputs(q, k, v, proj_w):
    """Build the 8 per-core input maps from the full tensors."""
    q = np.asarray(q, dtype=np.float32)
    k = np.asarray(k, dtype=np.float32)
    v = np.asarray(v, dtype=np.float32)
    proj_w = np.asarray(proj_w, dtype=np.float32)
    b_, n_, c_ = q.shape
    h_ = k.shape[1]
    d_ = c_ // h_
    nj = n_ // 128
    _np_dt = {"f32": np.float32, "f32r": np.float32, "bf16": ml_dtypes.bfloat16,
              "f16": np.float16}
    qk_np = _np_dt[os.environ.get("ATTN_KERNEL_QK_DT", "f16")]
    qh = np.ascontiguousarray(
        q.reshape(b_, n_, h_, d_).transpose(0, 2, 3, 1).astype(qk_np)
    )
    kh = np.ascontiguousarray(k.transpose(0, 1, 3, 2).astype(qk_np))
    in_maps = []
    for c in range(NCORES):
        b = c // 4
        hh0 = HPC * (c % 4)
        qt = np.ascontiguousarray(qh[b, hh0 : hh0 + HPC].reshape(NPAIR, 128, n_))
        kt = np.ascontiguousarray(kh[b, hh0 : hh0 + HPC].reshape(NPAIR, 128, n_))
        avd = os.environ.get("ATTN_KERNEL_AV_DT", "bf16")
        vp_np = ml_dtypes.bfloat16 if avd == "bf16" else np.float32
        vp = np.zeros((HPC, 128, nj, 128), vp_np)
        for hh in range(HPC):
            vv = v[b, hh0 + hh].reshape(nj, 128, d_).transpose(1, 0, 2)
            if hh % 2 == 0:
                vp[hh][:, :, 0:64] = vv
                vp[hh][:, :, 64] = 1.0
            else:
                vp[hh][:, :, 64:128] = vv
                vp[hh][:, :, 32] = 1.0
        ch0 = hh0 * d_
        pj_np = _np_dt[os.environ.get("ATTN_KERNEL_PJ_DT", "f16")]
        wt = np.ascontiguousarray(
            proj_w[:, ch0 : ch0 + HPC * d_].T.reshape(NPAIR, 128, c_).astype(pj_np)
        )
        in_maps.append({"qt": qt, "kt": kt, "vp": vp, "wt": wt})
    return in_maps


def reduce_outputs(results, proj_b):
    """Sum the per-core partial projections per batch and add the bias."""
    outs = [np.asarray(r["out"], dtype=np.float32) for r in results]
    full = np.stack(
        [outs[0] + outs[1] + outs[2] + outs[3], outs[4] + outs[5] + outs[6] + outs[7]]
    )
    return (full + np.asarray(proj_b, dtype=np.float32)[None, None, :]).astype(
        np.float32
    )


_NC_CACHE = {}


def _get_module():
    if "nc" not in _NC_CACHE:
        _NC_CACHE["nc"] = build_module()
    return _NC_CACHE["nc"]


def kernel(q, k, v, proj_w, proj_b):
    nc = _get_module()
    in_maps = shard_inputs(q, k, v, proj_w)
    trace = bool(int(os.environ.get("ATTN_KERNEL_TRACE", "0")))
    kwargs = {}
    tmpdir = os.environ.get("ATTN_KERNEL_TMPDIR")
    if trace and tmpdir:
        os.makedirs(tmpdir, exist_ok=True)
        kwargs["tmpdir"] = tmpdir
    res = bass_utils.run_bass_kernel_spmd(
        nc, in_maps, core_ids=list(range(NCORES)), trace=trace, **kwargs
    )
    if trace:
        _NC_CACHE["last_results"] = res
    return reduce_outputs(res.results, proj_b)
